# revision 7
# baseline (speedup 1.0000x reference)
"""TRN2 Bass kernel for nn_CosClassifier: sim = 10*scalar * cos_sim(inputs, proto).

Data-parallel over 8 NeuronCores: each core computes a (2048, 4096) slab of the
(16384, 4096) similarity matrix. Per core:
  1. DMA in x-slab (2048,256) in 4x512KB subgroups (emitted first so PE fills
     early), proto (4096,256) in 8x512KB subgroups, scalar.
  2. Per subgroup as it lands: row norms (ACT Square+accum -> Sqrt, DVE
     reciprocal), row scaling (x by 10/||x||, proto by scalar/||p||), then
     PE-transpose 128x128-blockwise; the PSUM->SBUF copy casts to float32r
     (TF32-like) so the main matmul runs at 1 cycle/row.
  3. dots matmul in fp32r (k-alternating lhsT; same-lhsT b2b fp32r is
     pathologically slow), fp32 accumulate in PSUM; PSUM->SBUF drains split
     ACT/DVE; 1MB contiguous half-row DMAs out.
"""
import sys

sys.path.insert(0, "/opt/trn_rl_repo")

import numpy as np

B, C, D = 16384, 4096, 256
NCORES = 8
BS = B // NCORES          # 2048 rows per core
NB = BS // 128            # 16 b-tiles per core
NCT = C // 128            # 32 c-tiles (proto rows)
NK = D // 128             # 2 k-tiles
NN = C // 512             # 8 n-blocks of 512
SGT = 4                   # tiles per subgroup (512KB)
XSG = NB // SGT           # 4 x subgroups
PSG = NCT // SGT          # 8 proto subgroups

_compiled = None


def _build():
    import concourse.bacc as bacc
    import concourse.mybir as mybir
    import concourse.tile as tile

    f32 = mybir.dt.float32
    f32r = mybir.dt.float32r
    Act = mybir.ActivationFunctionType

    nc = bacc.Bacc("TRN2", target_bir_lowering=False, debug=False,
                   num_devices=NCORES)

    x_d = nc.dram_tensor("x", [BS, D], f32, kind="ExternalInput").ap()
    p_d = nc.dram_tensor("proto", [C, D], f32, kind="ExternalInput").ap()
    s_d = nc.dram_tensor("scalar", [1, 1], f32, kind="ExternalInput").ap()
    id_d = nc.dram_tensor("identity", [128, 128], f32, kind="ExternalInput").ap()
    out_d = nc.dram_tensor("out", [BS, C], f32, kind="ExternalOutput").ap()

    with tile.TileContext(nc) as tc:
        with tc.tile_pool(name="sbuf", bufs=1) as pool, \
             tc.tile_pool(name="outp", bufs=4) as outp, \
             tc.tile_pool(name="psum_t", bufs=2, space="PSUM") as psum_t, \
             tc.tile_pool(name="psum_m", bufs=6, space="PSUM") as psum_m:

            x_r = x_d.rearrange("(n p) d -> p n d", p=128)       # [128, NB, 256]
            p_r = p_d.rearrange("(n p) d -> p n d", p=128)       # [128, NCT, 256]

            # ---- subgroup loads; x first so PE can start on x transposes ----
            xsg = []
            for g in range(XSG):
                t = pool.tile([128, SGT * D], f32, tag=f"xsg{g}")
                nc.sync.dma_start(
                    t[:].rearrange("p (n d) -> p n d", d=D),
                    x_r[:, g * SGT:(g + 1) * SGT, :])
                xsg.append(t)
            psg = []
            for g in range(PSG):
                t = pool.tile([128, SGT * D], f32, tag=f"psg{g}")
                nc.sync.dma_start(
                    t[:].rearrange("p (n d) -> p n d", d=D),
                    p_r[:, g * SGT:(g + 1) * SGT, :])
                psg.append(t)

            ident = pool.tile([128, 128], f32, tag="ident")
            nc.sync.dma_start(ident[:], id_d[:, :])
            sc = pool.tile([1, 1], f32, tag="sc")
            nc.sync.dma_start(sc[:], s_d[:, :])
            sc_b = pool.tile([128, 1], f32, tag="sc_b")
            nc.gpsimd.partition_broadcast(sc_b[:], sc[:])

            # transposed operands (f32r)
            # xt: k-block k at cols k*BS, b-tile i at +i*128
            xt = pool.tile([128, NK * BS], f32r, tag="xt")
            # pt: k-block k at cols k*C, c-tile j at +j*128
            pt = pool.tile([128, NK * C], f32r, tag="pt")

            cast_flip = [0]

            def process_subgroup(grp, gi, with_scalar, dst, dst_stride):
                for t in range(SGT):
                    src = grp[:, t * D:(t + 1) * D]
                    ssq = pool.tile([128, 1], f32, tag=f"ssq{t % 2}")
                    sq_scr = pool.tile([128, D], f32, tag=f"sqscr{t % 2}")
                    nc.scalar.activation(sq_scr[:], src, Act.Square,
                                         accum_out=ssq[:])
                    nrm = pool.tile([128, 1], f32, tag=f"nrm{t % 2}")
                    # x: sqrt(ssq)/10 (folds *10); proto: plain norm
                    nc.scalar.activation(nrm[:], ssq[:], Act.Sqrt,
                                         scale=1.0 if with_scalar else 0.01)
                    inv = pool.tile([128, 1], f32, tag=f"inv{t % 2}")
                    nc.vector.reciprocal(inv[:], nrm[:])
                    if with_scalar:
                        nc.vector.tensor_mul(inv[:], inv[:], sc_b[:])
                    nc.vector.tensor_scalar_mul(src, src, inv[:])
                for t in range(SGT):
                    gt = gi * SGT + t   # global tile index
                    for k in range(NK):
                        tp = psum_t.tile([128, 128], f32, tag="tp")
                        nc.tensor.transpose(
                            tp[:],
                            grp[:, t * D + k * 128: t * D + (k + 1) * 128],
                            ident[:])
                        cdst = dst[:, k * dst_stride + gt * 128:
                                   k * dst_stride + (gt + 1) * 128]
                        # casts: 2 of 3 on DVE, 1 of 3 on ACT
                        if cast_flip[0] % 3 == 2:
                            nc.scalar.copy(cdst, tp[:])
                        else:
                            nc.vector.tensor_copy(cdst, tp[:])
                        cast_flip[0] += 1

            # process in DMA arrival order: all x subgroups, then proto
            for g in range(XSG):
                process_subgroup(xsg[g], g, False, xt, BS)
            for g in range(PSG):
                process_subgroup(psg[g], g, True, pt, C)

            # ---- main matmul + drain ----
            drain_flip = [0]
            for i in range(NB):
                oh0 = outp.tile([128, C // 2], f32, tag="oh0")
                oh1 = outp.tile([128, C // 2], f32, tag="oh1")
                oh = [oh0, oh1]
                for n in range(NN):
                    ps = psum_m.tile([128, 512], f32, tag="mm")
                    for k in range(NK):
                        nc.tensor.matmul(
                            ps[:],
                            xt[:, k * BS + i * 128: k * BS + (i + 1) * 128],
                            pt[:, k * C + n * 512: k * C + (n + 1) * 512],
                            start=(k == 0), stop=(k == NK - 1))
                    dst = oh[n // 4][:, (n % 4) * 512:(n % 4 + 1) * 512]
                    # drains: 9 of 16 on ACT, 7 of 16 on DVE
                    if (drain_flip[0] * 9) % 16 < 9:
                        nc.scalar.copy(dst, ps[:])
                    else:
                        nc.vector.tensor_copy(dst, ps[:])
                    drain_flip[0] += 1
                for h in range(2):
                    nc.sync.dma_start(
                        out_d[i * 128:(i + 1) * 128,
                              h * (C // 2):(h + 1) * (C // 2)], oh[h][:])

    nc.compile()
    return nc


def _get_compiled():
    global _compiled
    if _compiled is None:
        _compiled = _build()
    return _compiled


def kernel(inputs, proto, scalar, _trace=False, **_tr_kw):
    from concourse.bass_utils import run_bass_kernel_spmd

    nc = _get_compiled()
    inputs = np.ascontiguousarray(inputs, dtype=np.float32)
    proto = np.ascontiguousarray(proto, dtype=np.float32)
    sc = np.asarray(scalar, dtype=np.float32).reshape(1, 1)
    ident = np.eye(128, dtype=np.float32)

    in_maps = []
    for c in range(NCORES):
        in_maps.append({
            "x": inputs[c * BS:(c + 1) * BS],
            "proto": proto,
            "scalar": sc,
            "identity": ident,
        })
    res = run_bass_kernel_spmd(nc, in_maps, core_ids=list(range(NCORES)),
                               trace=_trace, **_tr_kw)
    out = np.concatenate([res.results[c]["out"] for c in range(NCORES)], axis=0)
    if _trace:
        kernel.last_results = res
    return out
